# revision 7
# baseline (speedup 1.0000x reference)
"""Two-layer GCN (AttributeDecoder) as a distributed Bass kernel on 8 TRN2 NeuronCores.

Math (per reference):
    dis = (deg of A+I)^-1/2
    L1:  relu1 = relu( D @ ((A+I) @ (D @ x)) @ W1 + b1 )   with D = diag(dis)
    L2:  out   = relu( D @ ((A+I) @ (D @ relu1)) @ W2 + b2 )
using (A_hat @ h) @ W == A_hat @ (h @ W) so both layers aggregate 64-wide
features before the dense W matmul.

Sharding: destination nodes (and their in-edges) are partitioned contiguously
across the 8 cores. Each core aggregates messages for its own 1/8 of nodes,
gathering source rows from a replicated HBM feature table via dma_gather
(int16 indices; rows are fetched at 512B stride with an even/odd parity split
so indices fit int16). The layer-1 table (x * dis) is built on-device on every
core from the full x; the layer-2 table (relu1 * dis) is exchanged with one
AllGather. Tables are bf16 padded to 128 cols (256B rows, the dma_gather
minimum).

Per destination block of 128 nodes, edges (sorted by destination) are
processed in subtiles of 128: a gathered message tile [128 edges, 64 feats] is
combined with an on-device-built 0/1 selection matrix S [128 edges, 128 nodes]
(S[e, n] = 1 iff dst(e) == n) via TensorE matmuls accumulating in PSUM:
    aggT[f, n] += msg[e, f] * S[e, n]
then the dense W matmul, dis scaling, bias and relu per block.
"""

import os
import numpy as np
import ml_dtypes
_KDBG = set(os.environ.get('KDBG','').split(','))

from concourse import bass, mybir, bacc
import concourse.tile as tile
from concourse.bass_utils import run_bass_kernel_spmd

BF16 = ml_dtypes.bfloat16
P = 128
N_CORES = 8
FPAD = 128          # table row width (bf16) -> 256B rows
G = 4               # dst blocks per gather group


def _preprocess(x, edge_index, W1, b1, W2, b2):
    n = x.shape[0]
    f1 = x.shape[1]
    f2 = W2.shape[1]
    assert n % N_CORES == 0
    nsh = n // N_CORES
    nb = (nsh + P - 1) // P
    nsh_pad = nb * P
    assert nsh % 2 == 0 and nsh_pad % 2 == 0

    ei = np.asarray(edge_index).astype(np.int64)
    loops = np.arange(n, dtype=np.int64)
    src = np.concatenate([ei[0], loops])
    dst = np.concatenate([ei[1], loops])

    deg = np.bincount(dst, minlength=n).astype(np.float32)
    dis = np.where(deg > 0, 1.0 / np.sqrt(np.maximum(deg, 1e-12)), 0.0).astype(
        np.float32
    )

    owner = dst // nsh
    dloc = dst - owner * nsh
    blk = dloc // P
    par = (src % 2).astype(np.int64)     # same parity split works for both
    # layers: row2 = owner*nsh_pad + local keeps src's parity (both even).

    cnt = np.zeros((N_CORES, nb, 2), dtype=np.int64)
    np.add.at(cnt, (owner, blk, par), 1)
    T2 = int((cnt.max() + P - 1) // P)   # subtiles per (block, parity)
    TS = 2 * T2                          # subtile slots per block

    ntt = (n + P - 1) // P
    n_pad = ntt * P

    src2_row = (src // nsh) * nsh_pad + (src - (src // nsh) * nsh)

    n_groups = (nb + G - 1) // G

    def wrap16(flat):
        # dma_gather index image: item i -> [i % 16, i // 16], replicated to
        # all 8 16-partition groups.
        cols = len(flat) // 16
        img = flat.reshape(cols, 16).T            # [16, cols]
        return np.tile(img, (8, 1)).astype(np.int16)

    in_maps = []
    for c in range(N_CORES):
        m = owner == c
        s_c = src[m]
        s2_c = src2_row[m]
        b_c = blk[m]
        p_c = dloc[m] - b_c * P
        g_c = b_c * 2 + par[m]                    # (block, parity) group id

        order = np.argsort(g_c, kind="stable")
        s_c, s2_c, b_c, p_c, g_c = (
            s_c[order], s2_c[order], b_c[order], p_c[order], g_c[order]
        )
        cnt_c = cnt[c].reshape(-1)                # [nb*2]
        start = np.zeros(nb * 2, dtype=np.int64)
        start[1:] = np.cumsum(cnt_c)[:-1]
        slot = np.arange(len(g_c)) - start[g_c]
        lin = g_c * (T2 * P) + slot               # flat (block,parity,sub,edge)

        # per-block metadata in block-major slot order [nb, 2*T2, 128]
        src1h = np.zeros(nb * TS * P, dtype=np.int64)
        src2h = np.zeros(nb * TS * P, dtype=np.int64)
        dstl = np.full(nb * TS * P, -1, dtype=np.int16)
        src1h[lin] = s_c >> 1
        src2h[lin] = s2_c >> 1
        dstl[lin] = p_c

        src1h = src1h.reshape(nb, TS, P)
        src2h = src2h.reshape(nb, TS, P)

        # gather-call order: per group g: [even subtiles of its blocks],
        # [odd subtiles of its blocks]
        def call_order(a):
            segs = []
            for g in range(n_groups):
                g0, g1 = g * G, min(g * G + G, nb)
                segs.append(a[g0:g1, :T2].reshape(-1, P))
                segs.append(a[g0:g1, T2:].reshape(-1, P))
            return np.concatenate(segs).reshape(-1)

        src1_img = wrap16(call_order(src1h))
        src2_img = wrap16(call_order(src2h))
        dstl = dstl.reshape(nb * TS, P).T.copy()  # SBUF image [P, nb*TS]

        dis_col = np.zeros((P, nb), dtype=np.float32)
        own = np.pad(dis[c * nsh : (c + 1) * nsh], (0, nsh_pad - nsh))
        dis_col[:, :] = own.reshape(nb, P).T

        in_maps.append(
            {"src1": src1_img, "src2": src2_img, "dstl": dstl,
             "dis_col": dis_col}
        )

    xf = np.zeros((n_pad, f1), dtype=np.float32)
    xf[:n] = np.asarray(x, dtype=np.float32)
    dis_pbt = np.zeros((P, ntt), dtype=np.float32)
    dis_pbt[:, :] = np.pad(dis, (0, n_pad - n)).reshape(ntt, P).T
    shared = {
        "xf": xf,
        "dis_pbt": dis_pbt,
        "w1": np.asarray(W1, dtype=np.float32).astype(BF16),
        "w2": np.asarray(W2, dtype=np.float32).astype(BF16),
        "b1b": np.tile(np.asarray(b1, dtype=np.float32), (P, 1)),
        "b2b": np.tile(np.asarray(b2, dtype=np.float32), (P, 1)),
    }
    for m in in_maps:
        m.update(shared)

    cfg = dict(n=n, f1=f1, f2=f2, nsh=nsh, nb=nb, nsh_pad=nsh_pad, T2=T2,
               TS=TS, ntt=ntt, n_pad=n_pad, n_groups=n_groups)
    return in_maps, cfg


def _bcast_mid(ap, t_sz, inner):
    """[P, inner] AP -> [P, (0, t_sz), inner]"""
    dims = [list(ap.ap[0]), [0, t_sz], list(ap.ap[1])]
    assert ap.ap[1][1] == inner
    return bass.AP(ap.tensor, ap.offset, dims)


def _pair_ap(handle, parity, n_rows, fpad):
    """view table [n_rows, fpad] as rows of one parity: item k -> row 2k+parity"""
    ap = handle.ap()
    return bass.AP(ap.tensor, parity * fpad, [[2 * fpad, n_rows // 2], [1, fpad]])


def _build(cfg):
    n_pad, ntt, nb, T2, TS = (cfg[k] for k in ("n_pad", "ntt", "nb", "T2", "TS"))
    f1, f2, nsh_pad, n_groups = (cfg[k] for k in ("f1", "f2", "nsh_pad", "n_groups"))
    dt = mybir.dt
    idx_cols = nb * TS * P // 16

    nc = bacc.Bacc("TRN2", target_bir_lowering=False, debug=False,
                   num_devices=N_CORES)

    xf = nc.dram_tensor("xf", [n_pad, f1], dt.float32, kind="ExternalInput")
    dis_pbt = nc.dram_tensor("dis_pbt", [P, ntt], dt.float32, kind="ExternalInput")
    w1 = nc.dram_tensor("w1", [f1, f1], dt.bfloat16, kind="ExternalInput")
    w2 = nc.dram_tensor("w2", [f1, f2], dt.bfloat16, kind="ExternalInput")
    b1b = nc.dram_tensor("b1b", [P, f1], dt.float32, kind="ExternalInput")
    b2b = nc.dram_tensor("b2b", [P, f2], dt.float32, kind="ExternalInput")
    src1 = nc.dram_tensor("src1", [P, idx_cols], dt.int16, kind="ExternalInput")
    src2 = nc.dram_tensor("src2", [P, idx_cols], dt.int16, kind="ExternalInput")
    dstl = nc.dram_tensor("dstl", [P, nb * TS], dt.int16, kind="ExternalInput")
    dis_col = nc.dram_tensor("dis_col", [P, nb], dt.float32, kind="ExternalInput")
    out = nc.dram_tensor("out", [nsh_pad, f2], dt.float32, kind="ExternalOutput")

    xs_tab = nc.dram_tensor("xs_tab", [n_pad, FPAD], dt.bfloat16)
    r1s_own = nc.dram_tensor("r1s_own", [nsh_pad, FPAD], dt.bfloat16)
    r1s_full = nc.dram_tensor("r1s_full", [N_CORES * nsh_pad, FPAD], dt.bfloat16,
                              addr_space="Shared")

    TCH = 32     # x-table tiles per build chunk

    with tile.TileContext(nc) as tc:
        with (
            tc.tile_pool(name="const", bufs=1) as constp,
            tc.tile_pool(name="xload", bufs=2) as xloadp,
            tc.tile_pool(name="xsc", bufs=2) as xscp,
            tc.tile_pool(name="msg", bufs=2) as msgp,
            tc.tile_pool(name="smat", bufs=2) as smatp,
            tc.tile_pool(name="eplg", bufs=3) as eplgp,
            tc.tile_pool(name="acc", bufs=1) as accp,
            tc.tile_pool(name="ps1", bufs=2, space="PSUM") as ps1p,
            tc.tile_pool(name="ps2", bufs=2, space="PSUM") as ps2p,
        ):
            # ---- constants ----
            iota_t = constp.tile([P, P], dt.int16)
            nc.gpsimd.iota(iota_t[:], pattern=[[1, P]], base=0,
                           channel_multiplier=0)
            w1_sb = constp.tile([f1, f1], dt.bfloat16)
            nc.sync.dma_start(out=w1_sb[:], in_=w1.ap())
            w2_sb = constp.tile([f1, f2], dt.bfloat16)
            nc.sync.dma_start(out=w2_sb[:], in_=w2.ap())
            b1_sb = constp.tile([P, f1], dt.float32)
            nc.sync.dma_start(out=b1_sb[:], in_=b1b.ap())
            b2_sb = constp.tile([P, f2], dt.float32)
            nc.sync.dma_start(out=b2_sb[:], in_=b2b.ap())
            dis_col_sb = constp.tile([P, nb], dt.float32)
            nc.sync.dma_start(out=dis_col_sb[:], in_=dis_col.ap())
            dis_pbt_sb = constp.tile([P, ntt], dt.float32)
            nc.sync.dma_start(out=dis_pbt_sb[:], in_=dis_pbt.ap())
            src1_sb = constp.tile([P, idx_cols], dt.int16)
            nc.sync.dma_start(out=src1_sb[:], in_=src1.ap())
            src2_sb = constp.tile([P, idx_cols], dt.int16)
            nc.sync.dma_start(out=src2_sb[:], in_=src2.ap())
            dstl_sb = constp.tile([P, nb * TS], dt.int16)
            nc.sync.dma_start(out=dstl_sb[:], in_=dstl.ap())

            # ---- phase A: layer-1 table  xs = bf16(x * dis), zero-padded ----
            xf_r = xf.ap().rearrange("(t p) f -> p t f", p=P)
            xs_r = xs_tab.ap().rearrange("(t p) f -> p t f", p=P)
            for c0 in range(0, ntt, TCH):
                c1 = min(c0 + TCH, ntt)
                ct = c1 - c0
                xt = xloadp.tile([P, TCH, f1], dt.float32, tag="xload")
                nc.sync.dma_start(out=xt[:, :ct, :], in_=xf_r[:, c0:c1, :])
                xs_t = xscp.tile([P, TCH, FPAD], dt.bfloat16, tag="xsc")
                nc.vector.memset(xs_t[:, :ct, f1:], 0.0)
                nc.vector.tensor_tensor(
                    out=xs_t[:, :ct, :f1],
                    in0=xt[:, :ct, :],
                    in1=dis_pbt_sb[:, c0:c1].to_broadcast([P, ct, f1]),
                    op=mybir.AluOpType.mult,
                )
                nc.sync.dma_start(out=xs_r[:, c0:c1, :], in_=xs_t[:, :ct, :])

            # ---- shared aggregation layer ----
            def layer(tab, tab_rows, src_sb, w_sb, b_sb, fo, emit):
                slot_base = 0
                for g in range(n_groups):
                    g0, g1 = g * G, min(g * G + G, nb)
                    gb = g1 - g0
                    half = gb * T2
                    msg = msgp.tile([P, G * TS, FPAD], dt.bfloat16, tag="msg")
                    for parity in range(2):
                        if 'nogather' in _KDBG:
                            nc.vector.memset(msg[:, parity*half:parity*half+half, :], 1.0)
                            continue
                        i0 = (slot_base + parity * half) * P
                        nc.gpsimd.dma_gather(
                            out_ap=msg[:, parity * half : parity * half + half, :],
                            in_ap=_pair_ap(tab, parity, tab_rows, FPAD),
                            idxs_ap=src_sb[:, i0 // 16 : (i0 + half * P) // 16],
                            num_idxs=half * P,
                            num_idxs_reg=half * P,
                            elem_size=FPAD,
                            elem_step=2 * FPAD,
                            single_packet=False,
                        )
                    for j, b in enumerate(range(g0, g1)):
                        sm = smatp.tile([P, TS, P], dt.bfloat16, tag="smat")
                        nc.vector.tensor_tensor(
                            out=sm[:],
                            in0=dstl_sb[:, b * TS : (b + 1) * TS].to_broadcast(
                                [P, TS, P]
                            ),
                            in1=_bcast_mid(iota_t[:], TS, P),
                            op=mybir.AluOpType.is_equal,
                        )
                        ps1 = ps1p.tile([f1, P], dt.float32, space="PSUM",
                                        tag="ps1")
                        for t in range(TS):
                            parity, tsub = (0, t) if t < T2 else (1, t - T2)
                            slot = parity * half + j * T2 + tsub
                            nc.tensor.matmul(
                                out=ps1[:],
                                lhsT=msg[:, slot, :f1],
                                rhs=sm[:, t, :],
                                start=(t == 0),
                                stop=(t == TS - 1),
                            )
                        aggT = eplgp.tile([f1, P], dt.bfloat16, tag="aggT")
                        nc.vector.tensor_copy(aggT[:], ps1[:])
                        ps2 = ps2p.tile([P, fo], dt.float32, space="PSUM",
                                        tag="ps2")
                        nc.tensor.matmul(out=ps2[:], lhsT=aggT[:], rhs=w_sb[:],
                                         start=True, stop=True)
                        tt = eplgp.tile([P, fo], dt.float32, tag="tt")
                        nc.vector.scalar_tensor_tensor(
                            out=tt[:],
                            in0=ps2[:],
                            scalar=dis_col_sb[:, b : b + 1],
                            in1=b_sb[:],
                            op0=mybir.AluOpType.mult,
                            op1=mybir.AluOpType.add,
                        )
                        emit(b, tt)
                    slot_base += gb * TS

            # ---- L1 ----
            r1s_sb = accp.tile([P, nb, FPAD], dt.bfloat16)
            pad_ap = bass.AP(
                r1s_sb[:].tensor, r1s_sb[:].offset + f1,
                [list(r1s_sb[:].ap[0]), [FPAD, nb], [1, FPAD - f1]],
            )
            nc.vector.memset(pad_ap, 0.0)

            def emit1(b, tt):
                nc.vector.scalar_tensor_tensor(
                    out=r1s_sb[:, b, :f1],
                    in0=tt[:],
                    scalar=0.0,
                    in1=dis_col_sb[:, b : b + 1].to_broadcast([P, f1]),
                    op0=mybir.AluOpType.max,
                    op1=mybir.AluOpType.mult,
                )

            layer(xs_tab, n_pad, src1_sb, w1_sb, b1_sb, f1, emit1)

            r1s_own_r = r1s_own.ap().rearrange("(b p) f -> p b f", p=P)
            nc.sync.dma_start(out=r1s_own_r, in_=r1s_sb[:])

            # ---- exchange scaled layer-1 output across cores ----
            if 'nocc' in _KDBG:
                nc.sync.dma_start(out=r1s_full.ap().rearrange("(c r) f -> c r f", c=N_CORES)[0], in_=r1s_own.ap())
            else:
                nc.gpsimd.collective_compute(
                    "AllGather",
                    mybir.AluOpType.bypass,
                    replica_groups=[list(range(N_CORES))],
                    ins=[r1s_own.ap().opt()],
                    outs=[r1s_full.ap().opt()],
                )

            # ---- L2 ----
            out_sb = accp.tile([P, nb, f2], dt.float32)

            def emit2(b, tt):
                nc.vector.tensor_scalar_max(out_sb[:, b, :], tt[:], 0.0)

            layer(r1s_full, N_CORES * nsh_pad, src2_sb, w2_sb, b2_sb, f2, emit2)

            out_r = out.ap().rearrange("(b p) f -> p b f", p=P)
            nc.sync.dma_start(out=out_r, in_=out_sb[:])

    nc.compile()
    return nc


_CACHE = {}


def kernel(x, edge_index, W1, b1, W2, b2, _want_profile=False):
    x = np.asarray(x)
    in_maps, cfg = _preprocess(x, edge_index, W1, b1, W2, b2)
    key = (cfg["n"], cfg["f1"], cfg["f2"], cfg["T2"])
    if key not in _CACHE:
        _CACHE[key] = _build(cfg)
    nc = _CACHE[key]
    res = run_bass_kernel_spmd(
        nc, in_maps, core_ids=list(range(N_CORES)), trace=_want_profile
    )
    nsh = cfg["nsh"]
    outs = [res.results[c]["out"][:nsh] for c in range(N_CORES)]
    full = np.concatenate(outs, axis=0).astype(np.float32)
    if _want_profile:
        return full, res
    return full


# revision 8
# speedup vs baseline: 1.6225x; 1.6225x over previous
"""Two-layer GCN (AttributeDecoder) as a distributed Bass kernel on 8 TRN2 NeuronCores.

Math (per reference):
    dis = (deg of A+I)^-1/2
    L1:  relu1 = relu( D @ ((A+I) @ (D @ x)) @ W1 + b1 )   with D = diag(dis)
    L2:  out   = relu( D @ ((A+I) @ (D @ relu1)) @ W2 + b2 )
using (A_hat @ h) @ W == A_hat @ (h @ W) so both layers aggregate 64-wide
features before the dense W matmul.

Sharding: destination nodes (and their in-edges) are partitioned contiguously
across the 8 cores. Each core aggregates messages for its own 1/8 of nodes,
gathering source rows from a replicated HBM feature table via dma_gather
(int16 indices; rows are fetched at 512B stride with an even/odd parity split
so indices fit int16). The layer-1 table (x * dis) is built on-device on every
core from the full x; the layer-2 table (relu1 * dis) is exchanged with one
AllGather. Tables are bf16 padded to 128 cols (256B rows, the dma_gather
minimum).

Per destination block of 128 nodes, edges (sorted by destination) are
processed in subtiles of 128: a gathered message tile [128 edges, 64 feats] is
combined with an on-device-built 0/1 selection matrix S [128 edges, 128 nodes]
(S[e, n] = 1 iff dst(e) == n) via TensorE matmuls accumulating in PSUM:
    aggT[f, n] += msg[e, f] * S[e, n]
then the dense W matmul, dis scaling, bias and relu per block.
"""

import os
import numpy as np
import ml_dtypes
_KDBG = set(os.environ.get('KDBG','').split(','))

from concourse import bass, mybir, bacc
import concourse.tile as tile
from concourse.bass_utils import run_bass_kernel_spmd

BF16 = ml_dtypes.bfloat16
P = 128
N_CORES = 8
FPAD = 128          # table row width (bf16) -> 256B rows
G = 4               # dst blocks per gather group


def _preprocess(x, edge_index, W1, b1, W2, b2):
    n = x.shape[0]
    f1 = x.shape[1]
    f2 = W2.shape[1]
    assert n % N_CORES == 0
    nsh = n // N_CORES
    nb = (nsh + P - 1) // P
    nsh_pad = nb * P
    assert nsh % 2 == 0 and nsh_pad % 2 == 0

    ei = np.asarray(edge_index).astype(np.int64)
    loops = np.arange(n, dtype=np.int64)
    src = np.concatenate([ei[0], loops])
    dst = np.concatenate([ei[1], loops])

    deg = np.bincount(dst, minlength=n).astype(np.float32)
    dis = np.where(deg > 0, 1.0 / np.sqrt(np.maximum(deg, 1e-12)), 0.0).astype(
        np.float32
    )

    owner = dst // nsh
    dloc = dst - owner * nsh
    blk = dloc // P
    par = (src % 2).astype(np.int64)     # same parity split works for both
    # layers: row2 = owner*nsh_pad + local keeps src's parity (both even).

    cnt = np.zeros((N_CORES, nb, 2), dtype=np.int64)
    np.add.at(cnt, (owner, blk, par), 1)
    T2 = int((cnt.max() + P - 1) // P)   # subtiles per (block, parity)
    TS = 2 * T2                          # subtile slots per block

    ntt = (n + P - 1) // P
    n_pad = ntt * P

    src2_row = (src // nsh) * nsh_pad + (src - (src // nsh) * nsh)

    n_groups = (nb + G - 1) // G

    def wrap16(flat):
        # dma_gather index image: item i -> [i % 16, i // 16], replicated to
        # all 8 16-partition groups.
        cols = len(flat) // 16
        img = flat.reshape(cols, 16).T            # [16, cols]
        return np.tile(img, (8, 1)).astype(np.int16)

    in_maps = []
    for c in range(N_CORES):
        m = owner == c
        s_c = src[m]
        s2_c = src2_row[m]
        b_c = blk[m]
        p_c = dloc[m] - b_c * P
        g_c = b_c * 2 + par[m]                    # (block, parity) group id

        order = np.argsort(g_c, kind="stable")
        s_c, s2_c, b_c, p_c, g_c = (
            s_c[order], s2_c[order], b_c[order], p_c[order], g_c[order]
        )
        cnt_c = cnt[c].reshape(-1)                # [nb*2]
        start = np.zeros(nb * 2, dtype=np.int64)
        start[1:] = np.cumsum(cnt_c)[:-1]
        slot = np.arange(len(g_c)) - start[g_c]
        lin = g_c * (T2 * P) + slot               # flat (block,parity,sub,edge)

        # per-block metadata in block-major slot order [nb, 2*T2, 128]
        src1h = np.zeros(nb * TS * P, dtype=np.int64)
        src2h = np.zeros(nb * TS * P, dtype=np.int64)
        dstl = np.full(nb * TS * P, -1, dtype=np.int16)
        src1h[lin] = s_c >> 1
        src2h[lin] = s2_c >> 1
        dstl[lin] = p_c

        src1h = src1h.reshape(nb, TS, P)
        src2h = src2h.reshape(nb, TS, P)

        # gather-call order: per group g: [even subtiles of its blocks],
        # [odd subtiles of its blocks]
        def call_order(a):
            segs = []
            for g in range(n_groups):
                g0, g1 = g * G, min(g * G + G, nb)
                segs.append(a[g0:g1, :T2].reshape(-1, P))
                segs.append(a[g0:g1, T2:].reshape(-1, P))
            return np.concatenate(segs).reshape(-1)

        src1_img = wrap16(call_order(src1h))
        src2_img = wrap16(call_order(src2h))
        dstl = dstl.reshape(nb * TS, P).T.copy()  # SBUF image [P, nb*TS]

        dis_col = np.zeros((P, nb), dtype=np.float32)
        own = np.pad(dis[c * nsh : (c + 1) * nsh], (0, nsh_pad - nsh))
        dis_col[:, :] = own.reshape(nb, P).T

        in_maps.append(
            {"src1": src1_img, "src2": src2_img, "dstl": dstl,
             "dis_col": dis_col}
        )

    xf = np.zeros((n_pad, f1), dtype=np.float32)
    xf[:n] = np.asarray(x, dtype=np.float32)
    dis_pbt = np.zeros((P, ntt), dtype=np.float32)
    dis_pbt[:, :] = np.pad(dis, (0, n_pad - n)).reshape(ntt, P).T
    shared = {
        "xf": xf,
        "dis_pbt": dis_pbt,
        "w1": np.asarray(W1, dtype=np.float32).astype(BF16),
        "w2": np.asarray(W2, dtype=np.float32).astype(BF16),
        "b1b": np.tile(np.asarray(b1, dtype=np.float32), (P, 1)),
        "b2b": np.tile(np.asarray(b2, dtype=np.float32), (P, 1)),
    }
    for m in in_maps:
        m.update(shared)

    cfg = dict(n=n, f1=f1, f2=f2, nsh=nsh, nb=nb, nsh_pad=nsh_pad, T2=T2,
               TS=TS, ntt=ntt, n_pad=n_pad, n_groups=n_groups)
    return in_maps, cfg


def _bcast_mid(ap, t_sz, inner):
    """[P, inner] AP -> [P, (0, t_sz), inner]"""
    dims = [list(ap.ap[0]), [0, t_sz], list(ap.ap[1])]
    assert ap.ap[1][1] == inner
    return bass.AP(ap.tensor, ap.offset, dims)


def _pair_ap(handle, parity, n_rows, fpad):
    """view table [n_rows, fpad] as rows of one parity: item k -> row 2k+parity"""
    ap = handle.ap()
    return bass.AP(ap.tensor, parity * fpad, [[2 * fpad, n_rows // 2], [1, fpad]])


def _build(cfg):
    n_pad, ntt, nb, T2, TS = (cfg[k] for k in ("n_pad", "ntt", "nb", "T2", "TS"))
    f1, f2, nsh_pad, n_groups = (cfg[k] for k in ("f1", "f2", "nsh_pad", "n_groups"))
    dt = mybir.dt
    idx_cols = nb * TS * P // 16

    nc = bacc.Bacc("TRN2", target_bir_lowering=False, debug=False,
                   num_devices=N_CORES, num_swdge_queues=3)

    xf = nc.dram_tensor("xf", [n_pad, f1], dt.float32, kind="ExternalInput")
    dis_pbt = nc.dram_tensor("dis_pbt", [P, ntt], dt.float32, kind="ExternalInput")
    w1 = nc.dram_tensor("w1", [f1, f1], dt.bfloat16, kind="ExternalInput")
    w2 = nc.dram_tensor("w2", [f1, f2], dt.bfloat16, kind="ExternalInput")
    b1b = nc.dram_tensor("b1b", [P, f1], dt.float32, kind="ExternalInput")
    b2b = nc.dram_tensor("b2b", [P, f2], dt.float32, kind="ExternalInput")
    src1 = nc.dram_tensor("src1", [P, idx_cols], dt.int16, kind="ExternalInput")
    src2 = nc.dram_tensor("src2", [P, idx_cols], dt.int16, kind="ExternalInput")
    dstl = nc.dram_tensor("dstl", [P, nb * TS], dt.int16, kind="ExternalInput")
    dis_col = nc.dram_tensor("dis_col", [P, nb], dt.float32, kind="ExternalInput")
    out = nc.dram_tensor("out", [nsh_pad, f2], dt.float32, kind="ExternalOutput")

    xs_tab = nc.dram_tensor("xs_tab", [n_pad, FPAD], dt.bfloat16)
    r1s_own = nc.dram_tensor("r1s_own", [nsh_pad, FPAD], dt.bfloat16)
    r1s_full = nc.dram_tensor("r1s_full", [N_CORES * nsh_pad, FPAD], dt.bfloat16,
                              addr_space="Shared")

    TCH = 32     # x-table tiles per build chunk

    with tile.TileContext(nc) as tc:
        with (
            tc.tile_pool(name="const", bufs=1) as constp,
            tc.tile_pool(name="xload", bufs=2) as xloadp,
            tc.tile_pool(name="xsc", bufs=2) as xscp,
            tc.tile_pool(name="msg", bufs=2) as msgp,
            tc.tile_pool(name="smat", bufs=2) as smatp,
            tc.tile_pool(name="eplg", bufs=3) as eplgp,
            tc.tile_pool(name="acc", bufs=1) as accp,
            tc.tile_pool(name="ps1", bufs=2, space="PSUM") as ps1p,
            tc.tile_pool(name="ps2", bufs=2, space="PSUM") as ps2p,
        ):
            # ---- constants ----
            iota_t = constp.tile([P, P], dt.int16)
            nc.gpsimd.iota(iota_t[:], pattern=[[1, P]], base=0,
                           channel_multiplier=0)
            w1_sb = constp.tile([f1, f1], dt.bfloat16)
            nc.sync.dma_start(out=w1_sb[:], in_=w1.ap())
            w2_sb = constp.tile([f1, f2], dt.bfloat16)
            nc.sync.dma_start(out=w2_sb[:], in_=w2.ap())
            b1_sb = constp.tile([P, f1], dt.float32)
            nc.sync.dma_start(out=b1_sb[:], in_=b1b.ap())
            b2_sb = constp.tile([P, f2], dt.float32)
            nc.sync.dma_start(out=b2_sb[:], in_=b2b.ap())
            dis_col_sb = constp.tile([P, nb], dt.float32)
            nc.sync.dma_start(out=dis_col_sb[:], in_=dis_col.ap())
            dis_pbt_sb = constp.tile([P, ntt], dt.float32)
            nc.sync.dma_start(out=dis_pbt_sb[:], in_=dis_pbt.ap())
            src1_sb = constp.tile([P, idx_cols], dt.int16)
            nc.sync.dma_start(out=src1_sb[:], in_=src1.ap())
            src2_sb = constp.tile([P, idx_cols], dt.int16)
            nc.sync.dma_start(out=src2_sb[:], in_=src2.ap())
            dstl_sb = constp.tile([P, nb * TS], dt.int16)
            nc.sync.dma_start(out=dstl_sb[:], in_=dstl.ap())

            # ---- phase A: layer-1 table  xs = bf16(x * dis), zero-padded ----
            xf_r = xf.ap().rearrange("(t p) f -> p t f", p=P)
            xs_r = xs_tab.ap().rearrange("(t p) f -> p t f", p=P)
            for c0 in range(0, ntt, TCH):
                c1 = min(c0 + TCH, ntt)
                ct = c1 - c0
                xt = xloadp.tile([P, TCH, f1], dt.float32, tag="xload")
                nc.sync.dma_start(out=xt[:, :ct, :], in_=xf_r[:, c0:c1, :])
                xs_t = xscp.tile([P, TCH, FPAD], dt.bfloat16, tag="xsc")
                nc.vector.memset(xs_t[:, :ct, f1:], 0.0)
                nc.vector.tensor_tensor(
                    out=xs_t[:, :ct, :f1],
                    in0=xt[:, :ct, :],
                    in1=dis_pbt_sb[:, c0:c1].to_broadcast([P, ct, f1]),
                    op=mybir.AluOpType.mult,
                )
                nc.sync.dma_start(out=xs_r[:, c0:c1, :], in_=xs_t[:, :ct, :])

            # ---- shared aggregation layer ----
            qctr = [0]

            def layer(tab, tab_rows, src_sb, w_sb, b_sb, fo, emit):
                slot_base = 0
                for g in range(n_groups):
                    g0, g1 = g * G, min(g * G + G, nb)
                    gb = g1 - g0
                    half = gb * T2
                    msg = msgp.tile([P, G * TS, FPAD], dt.bfloat16, tag="msg")
                    for parity in range(2):
                        if 'nogather' in _KDBG:
                            nc.vector.memset(msg[:, parity*half:parity*half+half, :], 1.0)
                            continue
                        i0 = (slot_base + parity * half) * P
                        nc.gpsimd.dma_gather(
                            out_ap=msg[:, parity * half : parity * half + half, :],
                            in_ap=_pair_ap(tab, parity, tab_rows, FPAD),
                            idxs_ap=src_sb[:, i0 // 16 : (i0 + half * P) // 16],
                            num_idxs=half * P,
                            num_idxs_reg=half * P,
                            elem_size=FPAD,
                            elem_step=2 * FPAD,
                            single_packet=False,
                            queue_num=qctr[0] % 3,
                        )
                        qctr[0] += 1
                    for j, b in enumerate(range(g0, g1)):
                        sm = smatp.tile([P, TS, P], dt.bfloat16, tag="smat")
                        nc.vector.tensor_tensor(
                            out=sm[:],
                            in0=dstl_sb[:, b * TS : (b + 1) * TS].to_broadcast(
                                [P, TS, P]
                            ),
                            in1=_bcast_mid(iota_t[:], TS, P),
                            op=mybir.AluOpType.is_equal,
                        )
                        ps1 = ps1p.tile([f1, P], dt.float32, space="PSUM",
                                        tag="ps1")
                        for t in range(TS):
                            parity, tsub = (0, t) if t < T2 else (1, t - T2)
                            slot = parity * half + j * T2 + tsub
                            nc.tensor.matmul(
                                out=ps1[:],
                                lhsT=msg[:, slot, :f1],
                                rhs=sm[:, t, :],
                                start=(t == 0),
                                stop=(t == TS - 1),
                            )
                        aggT = eplgp.tile([f1, P], dt.bfloat16, tag="aggT")
                        nc.vector.tensor_copy(aggT[:], ps1[:])
                        ps2 = ps2p.tile([P, fo], dt.float32, space="PSUM",
                                        tag="ps2")
                        nc.tensor.matmul(out=ps2[:], lhsT=aggT[:], rhs=w_sb[:],
                                         start=True, stop=True)
                        tt = eplgp.tile([P, fo], dt.float32, tag="tt")
                        nc.vector.scalar_tensor_tensor(
                            out=tt[:],
                            in0=ps2[:],
                            scalar=dis_col_sb[:, b : b + 1],
                            in1=b_sb[:],
                            op0=mybir.AluOpType.mult,
                            op1=mybir.AluOpType.add,
                        )
                        emit(b, tt)
                    slot_base += gb * TS

            # ---- L1 ----
            r1s_sb = accp.tile([P, nb, FPAD], dt.bfloat16)
            pad_ap = bass.AP(
                r1s_sb[:].tensor, r1s_sb[:].offset + f1,
                [list(r1s_sb[:].ap[0]), [FPAD, nb], [1, FPAD - f1]],
            )
            nc.vector.memset(pad_ap, 0.0)

            def emit1(b, tt):
                nc.vector.scalar_tensor_tensor(
                    out=r1s_sb[:, b, :f1],
                    in0=tt[:],
                    scalar=0.0,
                    in1=dis_col_sb[:, b : b + 1].to_broadcast([P, f1]),
                    op0=mybir.AluOpType.max,
                    op1=mybir.AluOpType.mult,
                )

            layer(xs_tab, n_pad, src1_sb, w1_sb, b1_sb, f1, emit1)

            r1s_own_r = r1s_own.ap().rearrange("(b p) f -> p b f", p=P)
            nc.sync.dma_start(out=r1s_own_r, in_=r1s_sb[:])

            # ---- exchange scaled layer-1 output across cores ----
            if 'nocc' in _KDBG:
                nc.sync.dma_start(out=r1s_full.ap().rearrange("(c r) f -> c r f", c=N_CORES)[0], in_=r1s_own.ap())
            else:
                nc.gpsimd.collective_compute(
                    "AllGather",
                    mybir.AluOpType.bypass,
                    replica_groups=[list(range(N_CORES))],
                    ins=[r1s_own.ap().opt()],
                    outs=[r1s_full.ap().opt()],
                )

            # ---- L2 ----
            out_sb = accp.tile([P, nb, f2], dt.float32)
            zeros_f2 = constp.tile([P, f2], dt.float32)
            nc.vector.memset(zeros_f2[:], 0.0)

            def emit2(b, tt):
                nc.vector.scalar_tensor_tensor(
                    out=out_sb[:, b, :],
                    in0=tt[:],
                    scalar=0.0,
                    in1=zeros_f2[:],
                    op0=mybir.AluOpType.max,
                    op1=mybir.AluOpType.add,
                )

            layer(r1s_full, N_CORES * nsh_pad, src2_sb, w2_sb, b2_sb, f2, emit2)

            out_r = out.ap().rearrange("(b p) f -> p b f", p=P)
            nc.sync.dma_start(out=out_r, in_=out_sb[:])

    nc.compile()
    return nc


_CACHE = {}


def kernel(x, edge_index, W1, b1, W2, b2, _want_profile=False):
    x = np.asarray(x)
    in_maps, cfg = _preprocess(x, edge_index, W1, b1, W2, b2)
    key = (cfg["n"], cfg["f1"], cfg["f2"], cfg["T2"])
    if key not in _CACHE:
        _CACHE[key] = _build(cfg)
    nc = _CACHE[key]
    res = run_bass_kernel_spmd(
        nc, in_maps, core_ids=list(range(N_CORES)), trace=_want_profile
    )
    nsh = cfg["nsh"]
    outs = [res.results[c]["out"][:nsh] for c in range(N_CORES)]
    full = np.concatenate(outs, axis=0).astype(np.float32)
    if _want_profile:
        return full, res
    return full
